# revision 14
# baseline (speedup 1.0000x reference)
"""Trainium2 Bass kernel for nn_DimNet (4D-conv net + pixel shuffle).

Math: the three 4D convs collapse to 2D convs over flattened angular dims:
  conv1:  in [25, 104, 104] -> out [400, 96, 96], 9x9 kernel        (bf16)
  conv2a: in [25, 104, 104] -> buf [180, 100, 100] (20ch x 3x3 ang) (bf16)
  conv2b: buf [180,100,100] -> out [400, 96, 96], 5x5 kernel        (fp8 DR)
  mid = (p1 + relu-path)/2; pixel-shuffle (host side, pure data movement)

conv1/conv2a: contraction K packed as (channel, kh-shift) on partitions,
kw handled by free-dim offsets, accumulated in PSUM across kw / K-chunks.

conv2b runs in fp8e4 DoubleRow (2 virtual K-rows per partition, 2 cols/
cycle): buf is stored as [128, 2, 101, 100] fp8 with plane0 = sigma 0..127
and plane1 = sigma 128..179 (pad lanes zero), so one DR matmul per (kh,kw)
contracts all 180 sigma channels.  The rhs AP flattens (rows x 100) into one
free dim (rows are stored contiguously at width 100), putting the kw shift
in the offset; output columns 96..99 are garbage and skipped by the drain.
w2b is pre-scaled by 32 (fp8 subnormal floor) and the drain applies 1/32.
Numerics (numpy sim): rel err ~1.2e-2 vs the 2e-2 gate.

Phase B loops weights outermost over groups of 8 row-blocks (8 PSUM banks)
so each explicitly loaded weight (ldweights + non-self-loading matmuls)
serves 8 consecutive matmuls: DoubleRow LDWEIGHTS is serial on the PE and
expensive, so amortizing it is worth ~100us/rep.  (The same restructuring
applied to the bf16 convs made them slower -- self-loading bf16 matmuls
pipeline their weight load for free; explicit ldweights does not.)

Sharding: batch (2) x output-channel chunk (4 x 100) = 8 cores. conv2a is
replicated per core (small); gather + pixel shuffle on host.
"""

import os
import time

import ml_dtypes
import numpy as np

import concourse.tile as tile
from concourse import bacc, mybir
from concourse.ap import AP

F32 = mybir.dt.float32
BF16 = mybir.dt.bfloat16
F8 = mybir.dt.float8e4

MM_DT = BF16
MM_NP = ml_dtypes.bfloat16
NP8 = ml_dtypes.float8_e4m3

B = 2
H = 96
W = 96
HP = H + 8  # 104
WP = W + 8  # 104
CO = 100   # output channels per core (400 / 4)
W2B_SCALE = 32.0  # fp8 range lift for w2b; drain divides it back out

_RUNNERS = {}


def _build_nc(reps=1):
    nc = bacc.Bacc("TRN2", target_bir_lowering=False, debug=False,
                   enable_asserts=True, num_devices=8)

    xk1 = nc.dram_tensor("xk1", [125, 100, WP], MM_DT, kind="ExternalInput").ap()
    xk2 = nc.dram_tensor("xk2", [100, 96, WP], MM_DT, kind="ExternalInput").ap()
    w1a = nc.dram_tensor("w1a", [125, 9, CO], MM_DT, kind="ExternalInput").ap()
    w1b = nc.dram_tensor("w1b", [100, 9, CO], MM_DT, kind="ExternalInput").ap()
    w2a = nc.dram_tensor("w2a", [125, 5, 180], MM_DT, kind="ExternalInput").ap()
    # conv2b weights, DoubleRow layout [sigma_lo, (kh,kw), plane, m] fp8:
    # plane0 = sigma 0..127, plane1 = sigma 128..179 (lanes 52.. zero),
    # m padded 100->128 so the plane stride is 16B-aligned
    w2bd = nc.dram_tensor("w2bd", [128, 25, 2, 128], F8,
                          kind="ExternalInput").ap()
    ba1 = nc.dram_tensor("ba1", [128, 1], F32, kind="ExternalInput").ap()
    ba2 = nc.dram_tensor("ba2", [52, 1], F32, kind="ExternalInput").ap()
    b1h = nc.dram_tensor("b1h", [CO, 1], F32, kind="ExternalInput").ap()
    b2bh = nc.dram_tensor("b2bh", [CO, 1], F32, kind="ExternalInput").ap()
    # one output per rep so no rep's work is dead (reps>1 is timing-only)
    outs_d = [nc.dram_tensor("out" if r == 0 else f"out{r}", [CO, H, W], F32,
                             kind="ExternalOutput").ap() for r in range(reps)]

    Relu = mybir.ActivationFunctionType.Relu
    Add = mybir.AluOpType.add
    DR = mybir.MatmulPerfMode.DoubleRow

    def mm(out, lhsT, rhs, start, stop, perf_mode=None):
        nc.tensor.matmul(out, lhsT, rhs, start=start, stop=stop,
                         perf_mode=perf_mode)

    from contextlib import ExitStack

    with tile.TileContext(nc) as tc:
        with (
            tc.tile_pool(name="const", bufs=1) as const,
            tc.tile_pool(name="tmp", bufs=3) as tmp,
            tc.tile_pool(name="outp", bufs=3) as outp,
        ):
          # weights/biases loaded once (shared across timing reps)
          w1a_t = const.tile([125, 9, CO], MM_DT)
          w1b_t = const.tile([100, 9, CO], MM_DT)
          w2a_t = const.tile([125, 5, 180], MM_DT)
          w2bd_t = const.tile([128, 25, 2, 128], F8)
          ba1_t = const.tile([128, 1], F32)
          ba2_t = const.tile([52, 1], F32)
          b1h_t = const.tile([CO, 1], F32)
          b2bh_t = const.tile([CO, 1], F32)
          # weights on the gpsimd queue so the streaming xk loads on the
          # sync queue aren't stuck behind the weight traffic
          # phase-A weights/biases first; the big conv2b weight tensor is
          # only needed in phase B so it loads last
          for t, src in ((w2a_t, w2a), (w1a_t, w1a), (w1b_t, w1b),
                         (ba1_t, ba1), (ba2_t, ba2), (b1h_t, b1h),
                         (b2bh_t, b2bh), (w2bd_t, w2bd)):
              nc.gpsimd.dma_start(out=t[:], in_=src)

          # fp8 DoubleRow buf: [sigma_lo, plane, row, col]; row 100 is a
          # scratch row so the flattened tail-block rhs stays in bounds.
          # Allocated once (pad lanes / scratch rows must be finite zeros;
          # each rep rewrites rows 0..99 before reading them).
          buf_t = const.tile([128, 2, 101, 100], F8)
          nc.gpsimd.memset(buf_t[:], 0.0)

          for _rep in range(reps):
            out = outs_d[_rep]
            _ph_a = ExitStack()
            xk1p = _ph_a.enter_context(
                tc.tile_pool(name=f"xk1p{_rep}", bufs=5))
            xk2p = _ph_a.enter_context(
                tc.tile_pool(name=f"xk2p{_rep}", bufs=5))
            psa = _ph_a.enter_context(
                tc.tile_pool(name=f"psa{_rep}", bufs=3, space="PSUM"))
            ps1p = _ph_a.enter_context(
                tc.tile_pool(name=f"ps1{_rep}", bufs=2, space="PSUM"))
            p1h_t = const.tile([CO, H, W], F32)

            # ---- Phase A: conv2a (20 5-row blocks) + conv1 (19 5-row + 1),
            # inputs streamed in 20-row macro-chunks (5 big DMAs per tensor)
            MCR = 20
            RB = 5
            for mc in range(5):
                m0 = mc * MCR
                xc1 = xk1p.tile([125, MCR, WP], MM_DT)
                nc.sync.dma_start(out=xc1[:], in_=xk1[:, m0:m0 + MCR, :])
                n2 = min(MCR, H - m0)
                if n2 > 0:
                    xc2 = xk2p.tile([100, MCR, WP], MM_DT)
                    nc.sync.dma_start(out=xc2[:, 0:n2, :],
                                      in_=xk2[:, m0:m0 + n2, :])

                for j in range(4):
                    r0 = m0 + j * RB  # global row
                    q0 = j * RB       # row within macro-chunk
                    # conv2a: out channels sigma=(a1',a2',c) in two M chunks
                    pa1 = psa.tile([128, RB, 100], F32, tag="pa")
                    for kw in range(5):
                        mm(pa1[:], w2a_t[:, kw, 0:128],
                           xc1[:, q0:q0 + RB, kw:kw + 100],
                           start=(kw == 0), stop=(kw == 4))
                    nc.scalar.activation(buf_t[:, 0, r0:r0 + RB, :], pa1[:],
                                         Relu, bias=ba1_t[:])
                    pa2 = psa.tile([52, RB, 100], F32, tag="pa")
                    for kw in range(5):
                        mm(pa2[:], w2a_t[:, kw, 128:180],
                           xc1[:, q0:q0 + RB, kw:kw + 100],
                           start=(kw == 0), stop=(kw == 4))
                    # chunk2 relu on DVE so ACT isn't the drain bottleneck
                    nc.vector.tensor_scalar(buf_t[0:52, 1, r0:r0 + RB, :],
                                            pa2[:], ba2_t[:], 0.0, Add,
                                            mybir.AluOpType.max)

                    # conv1 on the same 5-row grid (rows 0..94; row 95 below)
                    rr = min(RB, H - r0)
                    if rr > 0:
                        p1 = ps1p.tile([CO, RB, W], F32)
                        for kw in range(9):
                            mm(p1[:, 0:rr, :], w1a_t[:, kw, :],
                               xc1[:, q0:q0 + rr, kw:kw + W],
                               start=(kw == 0), stop=False)
                        for kw in range(9):
                            mm(p1[:, 0:rr, :], w1b_t[:, kw, :],
                               xc2[:, q0:q0 + rr, kw:kw + W],
                               start=False, stop=(kw == 8))
                        # w1/b1 pre-halved on host: p1h = psum + b1h
                        nc.vector.tensor_scalar_add(p1h_t[:, r0:r0 + rr, :],
                                                    p1[:, 0:rr, :], b1h_t[:])

            # phase-A psum/xk pools released -> conv2b gets 7 PSUM banks
            _ph_a.close()
            ps2p = ExitStack()
            ps2 = ps2p.enter_context(
                tc.tile_pool(name=f"ps2{_rep}", bufs=8, space="PSUM"))

            # ---- Phase B: conv2b, fp8 DoubleRow, weights outermost over
            # groups of 5 row-blocks (5 live PSUM accumulators)
            def buf_rhs(r, rr, kw):
                # [128, 2, rr*100] flat view of buf rows r..r+rr-1, col
                # offset kw; reads beyond row r+rr-1 land in later rows /
                # the scratch row and only feed garbage output columns
                sl = buf_t[:, :, r:r + rr, :]
                return AP(tensor=sl.tensor, offset=sl.offset + kw,
                          ap=[sl.ap[0], sl.ap[1], [1, rr * 100]])

            for g, (b0, nb) in enumerate(((0, 8), (8, 8), (16, 4))):
                h0s = [5 * (b0 + q) for q in range(nb)]
                rrs = [min(RB, H - h0) for h0 in h0s]
                p2s = [ps2.tile([128, RB, 100], F32, tag="p2",
                                name=f"p2_{g}_{q}") for q in range(nb)]
                for j in range(25):
                    kh, kw = divmod(j, 5)
                    # one explicit weight load per (kh,kw); the matmuls
                    # below are marked non-self-loading so the 256-column
                    # DoubleRow LDWEIGHTS is paid once, not nb times
                    nc.tensor.ldweights(w2bd_t[:, j], perf_mode=DR)
                    for q in range(nb):
                        h0, rr = h0s[q], rrs[q]
                        inst = nc.tensor.matmul(
                            p2s[q][:, 0:rr, :], w2bd_t[:, j],
                            buf_rhs(h0 + kh, rr, kw),
                            start=(j == 0), stop=(j == 24), perf_mode=DR)
                        raw = inst.ins if hasattr(inst, "ins") else inst
                        raw.ldweights = False
                for q in range(nb):
                    h0, rr = h0s[q], rrs[q]
                    # w2b/b2b pre-halved; w2b also x32 for fp8 -> scale back
                    tt = tmp.tile([CO, RB, W], F32)
                    nc.scalar.activation(tt[:, 0:rr, :],
                                         p2s[q][0:CO, 0:rr, 0:W], Relu,
                                         bias=b2bh_t[:], scale=1.0 / W2B_SCALE)
                    ot = outp.tile([CO, RB, W], F32)
                    nc.vector.tensor_add(ot[:, 0:rr, :], tt[:, 0:rr, :],
                                         p1h_t[:, h0:h0 + rr, :])
                    nc.scalar.dma_start(out=out[:, h0:h0 + rr, :],
                                        in_=ot[:, 0:rr, :])
            ps2p.close()

    nc.compile()
    return nc


def _prep_in_maps(pic, w1, b1, w2a, b2a, w2b, b2b):
    pic = np.asarray(pic, dtype=np.float32).reshape(B, 25, H, W)
    w1r = np.asarray(w1, dtype=np.float32).reshape(400, 25, 9, 9)
    b1 = np.asarray(b1, dtype=np.float32)
    w2a = np.asarray(w2a, dtype=np.float32)
    b2a = np.asarray(b2a, dtype=np.float32)
    w2b = np.asarray(w2b, dtype=np.float32)
    b2b = np.asarray(b2b, dtype=np.float32)

    xpad = np.full((B, 25, HP, WP), 0.5, dtype=np.float32)
    xpad[:, :, 4:4 + H, 4:4 + W] = pic
    # xk1[b, cin*5+kh, r, w] = xpad[b, cin, r+kh, w]   (kh 0..4, r 0..99)
    xk1 = np.stack([xpad[:, :, kh:kh + 100, :] for kh in range(5)],
                   axis=2).reshape(B, 125, 100, WP)
    # xk2[b, cin*4+kh', h, w] = xpad[b, cin, h+5+kh', w] (kh' 0..3, h 0..95)
    xk2 = np.stack([xpad[:, :, 5 + kh:5 + kh + 96, :] for kh in range(4)],
                   axis=2).reshape(B, 100, 96, WP)

    # W2A[p=(a1*5+a2)*5+kh, kw, m=a1'*60+a2'*20+c] = w2a[c,0,da1,da2,kh,kw]
    W2A = np.zeros((125, 5, 180), dtype=np.float32)
    for a1p in range(3):
        for a2p in range(3):
            m0 = a1p * 60 + a2p * 20
            for da1 in range(3):
                for da2 in range(3):
                    p0 = ((a1p + da1) * 5 + (a2p + da2)) * 5
                    W2A[p0:p0 + 5, :, m0:m0 + 20] = np.transpose(
                        w2a[:, 0, da1, da2, :, :], (1, 2, 0))
    ba_full = np.tile(b2a, 9).astype(np.float32)[:, None]  # [180,1]

    in_maps = []
    for core in range(8):
        b, cc = divmod(core, 4)
        co0 = cc * CO
        # w1, w2b (and their biases) pre-scaled by 0.5 so the (p1+p2)/2
        # average is folded into the matmuls.
        w1sl = 0.5 * w1r[co0:co0 + CO]  # [100, 25, 9, 9]
        W1A = np.ascontiguousarray(
            np.transpose(w1sl[:, :, 0:5, :], (1, 2, 3, 0)).reshape(125, 9, CO))
        W1B = np.ascontiguousarray(
            np.transpose(w1sl[:, :, 5:9, :], (1, 2, 3, 0)).reshape(100, 9, CO))
        w2bsl = 0.5 * w2b[co0:co0 + CO]  # [100, 20, 3, 3, 5, 5]
        # [sigma=(da1,da2,c), (kh,kw), m]
        W2B = np.transpose(w2bsl, (2, 3, 1, 4, 5, 0)).reshape(180, 25, CO)
        w2bd = np.zeros((128, 25, 2, 128), dtype=np.float32)
        w2bd[:, :, 0, 0:CO] = W2B_SCALE * W2B[0:128]
        w2bd[0:52, :, 1, 0:CO] = W2B_SCALE * W2B[128:180]
        in_maps.append({
            "xk1": np.ascontiguousarray(xk1[b].astype(MM_NP)),
            "xk2": np.ascontiguousarray(xk2[b].astype(MM_NP)),
            "w1a": W1A.astype(MM_NP),
            "w1b": W1B.astype(MM_NP),
            "w2a": np.ascontiguousarray(W2A.astype(MM_NP)),
            "w2bd": np.ascontiguousarray(w2bd.astype(NP8)),
            "ba1": np.ascontiguousarray(ba_full[:128]),
            "ba2": np.ascontiguousarray(ba_full[128:]),
            "b1h": np.ascontiguousarray((0.5 * b1[co0:co0 + CO])[:, None]),
            "b2bh": np.ascontiguousarray((0.5 * b2b[co0:co0 + CO])[:, None]),
        })
    return in_maps


def _get_runner(reps=1):
    """Build nc once per reps and return a cached jitted SPMD executor."""
    if reps in _RUNNERS:
        return _RUNNERS[reps]

    import jax
    from jax.experimental.shard_map import shard_map
    from jax.sharding import Mesh, NamedSharding, PartitionSpec

    from concourse import mybir as _mybir
    from concourse.bass2jax import (_bass_exec_p, install_neuronx_cc_hook,
                                    partition_id_tensor)

    nc = _build_nc(reps)
    install_neuronx_cc_hook()

    n_cores = 8
    partition_name = (nc.partition_id_tensor.name
                      if nc.partition_id_tensor else None)
    in_names, out_names, out_avals, zero_outs = [], [], [], []
    for alloc in nc.m.functions[0].allocations:
        if not isinstance(alloc, _mybir.MemoryLocationSet):
            continue
        name = alloc.memorylocations[0].name
        if alloc.kind == "ExternalInput":
            if name != partition_name:
                in_names.append(name)
        elif alloc.kind == "ExternalOutput":
            shape = tuple(alloc.tensor_shape)
            dtype = _mybir.dt.np(alloc.dtype)
            out_names.append(name)
            out_avals.append(jax.core.ShapedArray(shape, dtype))
            zero_outs.append(np.zeros((n_cores * shape[0],) + shape[1:], dtype))
    assert nc.dbg_addr is None
    n_params = len(in_names)
    all_names = in_names + out_names
    if partition_name is not None:
        all_names = all_names + [partition_name]

    def _body(*args):
        operands = list(args)
        if partition_name is not None:
            operands.append(partition_id_tensor())
        outs = _bass_exec_p.bind(
            *operands,
            out_avals=tuple(out_avals),
            in_names=tuple(all_names),
            out_names=tuple(out_names),
            lowering_input_output_aliases=(),
            sim_require_finite=True,
            sim_require_nnan=True,
            nc=nc,
        )
        return tuple(outs)

    devices = jax.devices()[:n_cores]
    mesh = Mesh(np.asarray(devices), ("core",))
    nspec = (PartitionSpec("core"),) * (n_params + len(out_names))
    sharded = jax.jit(
        shard_map(_body, mesh=mesh, in_specs=nspec,
                  out_specs=(PartitionSpec("core"),) * len(out_names)),
        keep_unused=True)
    sharding = NamedSharding(mesh, PartitionSpec("core"))

    class Runner:
        def put(self, in_maps):
            """Transfer inputs (+ zero output bufs) to the devices once."""
            concat_in = [
                np.concatenate([np.asarray(m[name]) for m in in_maps], axis=0)
                for name in in_names
            ]
            return [jax.device_put(x, sharding)
                    for x in concat_in + zero_outs]

        def exec_timed(self, dev_args):
            t0 = time.perf_counter()
            out_arrs = sharded(*dev_args)
            # one sync only: under axon each block_until_ready is a costly
            # RPC, and blocking any output waits for the whole execution
            out_arrs[0].block_until_ready()
            return out_arrs, time.perf_counter() - t0

        def __call__(self, in_maps):
            out_arrs, dt = self.exec_timed(self.put(in_maps))
            per_core = [
                {name: np.asarray(out_arrs[i]).reshape(
                    n_cores, *out_avals[i].shape)[c]
                 for i, name in enumerate(out_names)}
                for c in range(n_cores)
            ]
            return per_core, dt

    run = Runner()
    _RUNNERS[reps] = run
    return run


def kernel(pic, w1, b1, w2a, b2a, w2b, b2b):
    run = _get_runner()
    in_maps = _prep_in_maps(pic, w1, b1, w2a, b2a, w2b, b2b)
    results, _ = run(in_maps)

    mid = np.empty((B, 400, H, W), dtype=np.float32)
    for core in range(8):
        b, cc = divmod(core, 4)
        mid[b, cc * CO:(cc + 1) * CO] = results[core]["out"]
    # pixel shuffle r=4, then split 25 -> 5x5
    y = mid.reshape(B, 25, 4, 4, H, W).transpose(0, 1, 4, 2, 5, 3)
    return np.ascontiguousarray(y).reshape(B, 5, 5, H * 4, W * 4)


# revision 16
# speedup vs baseline: 1.9864x; 1.9864x over previous
"""Trainium2 Bass kernel for nn_DimNet (4D-conv net + pixel shuffle).

Math: the three 4D convs collapse to 2D convs over flattened angular dims:
  conv1:  in [25, 104, 104] -> out [400, 96, 96], 9x9 kernel        (bf16)
  conv2a: in [25, 104, 104] -> buf [180, 100, 100] (20ch x 3x3 ang) (bf16)
  conv2b: buf [180,100,100] -> out [400, 96, 96], 5x5 kernel        (fp8 DR)
  mid = (p1 + relu-path)/2; pixel-shuffle (host side, pure data movement)

conv1/conv2a: contraction K packed as (channel, kh-shift) on partitions,
kw handled by free-dim offsets, accumulated in PSUM across kw / K-chunks.

conv2b runs in fp8e4 DoubleRow (2 virtual K-rows per partition, 2 cols/
cycle): buf is stored as [128, 2, 101, 100] fp8 with plane0 = sigma 0..127
and plane1 = sigma 128..179 (pad lanes zero), so one DR matmul per (kh,kw)
contracts all 180 sigma channels.  The rhs AP flattens (rows x 100) into one
free dim (rows are stored contiguously at width 100), putting the kw shift
in the offset; output columns 96..99 are garbage and skipped by the drain.
w2b is pre-scaled by 32 (fp8 subnormal floor) and the drain applies 1/32.
Numerics (numpy sim): rel err ~1.2e-2 vs the 2e-2 gate.

Phase B loops weights outermost over groups of 8 row-blocks (8 PSUM banks)
so each explicitly loaded weight (ldweights + non-self-loading matmuls)
serves 8 consecutive matmuls: DoubleRow LDWEIGHTS is serial on the PE and
expensive, so amortizing it is worth ~100us/rep.  (The same restructuring
applied to the bf16 convs made them slower -- self-loading bf16 matmuls
pipeline their weight load for free; explicit ldweights does not.)

Sharding: batch (2) x output-channel chunk (4 x 100) = 8 cores. conv2a is
replicated per core (small); gather + pixel shuffle on host.
"""

import os
import time

import ml_dtypes
import numpy as np

import concourse.tile as tile
from concourse import bacc, mybir
from concourse.ap import AP

F32 = mybir.dt.float32
BF16 = mybir.dt.bfloat16
F8 = mybir.dt.float8e4

MM_DT = BF16
MM_NP = ml_dtypes.bfloat16
NP8 = ml_dtypes.float8_e4m3

B = 2
H = 96
W = 96
HP = H + 8  # 104
WP = W + 8  # 104
CO = 100   # output channels per core (400 / 4)
W2B_SCALE = 32.0  # fp8 range lift for w2b; drain divides it back out

_RUNNERS = {}


def _build_nc(reps=1):
    nc = bacc.Bacc("TRN2", target_bir_lowering=False, debug=False,
                   enable_asserts=True, num_devices=8)

    xk1 = nc.dram_tensor("xk1", [125, 100, WP], MM_DT, kind="ExternalInput").ap()
    xk2 = nc.dram_tensor("xk2", [100, 96, WP], MM_DT, kind="ExternalInput").ap()
    w1a = nc.dram_tensor("w1a", [125, 9, CO], MM_DT, kind="ExternalInput").ap()
    w1b = nc.dram_tensor("w1b", [100, 9, CO], MM_DT, kind="ExternalInput").ap()
    w2a = nc.dram_tensor("w2a", [125, 5, 180], MM_DT, kind="ExternalInput").ap()
    # conv2b weights, DoubleRow layout [sigma_lo, (kh,kw), plane, m] fp8:
    # plane0 = sigma 0..127, plane1 = sigma 128..179 (lanes 52.. zero),
    # m padded 100->128 so the plane stride is 16B-aligned
    w2bd = nc.dram_tensor("w2bd", [128, 25, 2, 128], F8,
                          kind="ExternalInput").ap()
    ba1 = nc.dram_tensor("ba1", [128, 1], F32, kind="ExternalInput").ap()
    ba2 = nc.dram_tensor("ba2", [52, 1], F32, kind="ExternalInput").ap()
    b1h = nc.dram_tensor("b1h", [CO, 1], F32, kind="ExternalInput").ap()
    b2bh = nc.dram_tensor("b2bh", [CO, 1], F32, kind="ExternalInput").ap()
    # one output per rep so no rep's work is dead (reps>1 is timing-only)
    outs_d = [nc.dram_tensor("out" if r == 0 else f"out{r}", [CO, H, W], F32,
                             kind="ExternalOutput").ap() for r in range(reps)]

    Relu = mybir.ActivationFunctionType.Relu
    Add = mybir.AluOpType.add
    DR = mybir.MatmulPerfMode.DoubleRow

    def mm(out, lhsT, rhs, start, stop, perf_mode=None):
        nc.tensor.matmul(out, lhsT, rhs, start=start, stop=stop,
                         perf_mode=perf_mode)

    from contextlib import ExitStack

    with tile.TileContext(nc) as tc:
        with (
            tc.tile_pool(name="const", bufs=1) as const,
            tc.tile_pool(name="tmp", bufs=3) as tmp,
            tc.tile_pool(name="outp", bufs=3) as outp,
        ):
          # weights/biases loaded once (shared across timing reps)
          w1a_t = const.tile([125, 9, CO], MM_DT)
          w1b_t = const.tile([100, 9, CO], MM_DT)
          w2a_t = const.tile([125, 5, 180], MM_DT)
          w2bd_t = const.tile([128, 25, 2, 128], F8)
          ba1_t = const.tile([128, 1], F32)
          ba2_t = const.tile([52, 1], F32)
          b1h_t = const.tile([CO, 1], F32)
          b2bh_t = const.tile([CO, 1], F32)
          # weights on the gpsimd queue so the streaming xk loads on the
          # sync queue aren't stuck behind the weight traffic
          # phase-A weights/biases first; the big conv2b weight tensor is
          # only needed in phase B so it loads last
          for t, src in ((w2a_t, w2a), (w1a_t, w1a), (w1b_t, w1b),
                         (ba1_t, ba1), (ba2_t, ba2), (b1h_t, b1h),
                         (b2bh_t, b2bh), (w2bd_t, w2bd)):
              nc.gpsimd.dma_start(out=t[:], in_=src)

          # fp8 DoubleRow buf: [sigma_lo, plane, row, col]; row 100 is a
          # scratch row so the flattened tail-block rhs stays in bounds.
          # Allocated once (pad lanes / scratch rows must be finite zeros;
          # each rep rewrites rows 0..99 before reading them).
          buf_t = const.tile([128, 2, 101, 100], F8)
          nc.gpsimd.memset(buf_t[:], 0.0)

          for _rep in range(reps):
            out = outs_d[_rep]
            _ph_a = ExitStack()
            xk1p = _ph_a.enter_context(
                tc.tile_pool(name=f"xk1p{_rep}", bufs=5))
            xk2p = _ph_a.enter_context(
                tc.tile_pool(name=f"xk2p{_rep}", bufs=5))
            psa = _ph_a.enter_context(
                tc.tile_pool(name=f"psa{_rep}", bufs=4, space="PSUM"))
            ps1p = _ph_a.enter_context(
                tc.tile_pool(name=f"ps1{_rep}", bufs=3, space="PSUM"))
            p1h_t = const.tile([CO, H, W], F32)

            # ---- Phase A: conv2a (20 5-row blocks) + conv1 (19 5-row + 1),
            # inputs streamed in 20-row macro-chunks (5 big DMAs per tensor)
            MCR = 20
            RB = 5
            for mc in range(5):
                m0 = mc * MCR
                xc1 = xk1p.tile([125, MCR, WP], MM_DT)
                n2 = min(MCR, H - m0)
                xc2 = xk2p.tile([100, MCR, WP], MM_DT)
                if mc == 0:
                    # slice the first chunk's DMA per row-block so block-0
                    # compute starts once 5 rows land, not all 20
                    # (single-shot startup latency)
                    for j4 in range(4):
                        s = slice(j4 * RB, (j4 + 1) * RB)
                        nc.sync.dma_start(out=xc1[:, s, :], in_=xk1[:, s, :])
                        nc.sync.dma_start(out=xc2[:, s, :], in_=xk2[:, s, :])
                else:
                    nc.sync.dma_start(out=xc1[:], in_=xk1[:, m0:m0 + MCR, :])
                    if n2 > 0:
                        nc.sync.dma_start(out=xc2[:, 0:n2, :],
                                          in_=xk2[:, m0:m0 + n2, :])

                for j in range(4):
                    r0 = m0 + j * RB  # global row
                    q0 = j * RB       # row within macro-chunk
                    # conv2a: out channels sigma=(a1',a2',c) in two M chunks
                    pa1 = psa.tile([128, RB, 100], F32, tag="pa")
                    for kw in range(5):
                        mm(pa1[:], w2a_t[:, kw, 0:128],
                           xc1[:, q0:q0 + RB, kw:kw + 100],
                           start=(kw == 0), stop=(kw == 4))
                    nc.scalar.activation(buf_t[:, 0, r0:r0 + RB, :], pa1[:],
                                         Relu, bias=ba1_t[:])
                    pa2 = psa.tile([52, RB, 100], F32, tag="pa")
                    for kw in range(5):
                        mm(pa2[:], w2a_t[:, kw, 128:180],
                           xc1[:, q0:q0 + RB, kw:kw + 100],
                           start=(kw == 0), stop=(kw == 4))
                    # chunk2 relu on DVE so ACT isn't the drain bottleneck
                    nc.vector.tensor_scalar(buf_t[0:52, 1, r0:r0 + RB, :],
                                            pa2[:], ba2_t[:], 0.0, Add,
                                            mybir.AluOpType.max)

                    # conv1 on the same 5-row grid (rows 0..94; row 95 below)
                    rr = min(RB, H - r0)
                    if rr > 0:
                        p1 = ps1p.tile([CO, RB, W], F32)
                        for kw in range(9):
                            mm(p1[:, 0:rr, :], w1a_t[:, kw, :],
                               xc1[:, q0:q0 + rr, kw:kw + W],
                               start=(kw == 0), stop=False)
                        for kw in range(9):
                            mm(p1[:, 0:rr, :], w1b_t[:, kw, :],
                               xc2[:, q0:q0 + rr, kw:kw + W],
                               start=False, stop=(kw == 8))
                        # w1/b1 pre-halved on host: p1h = psum + b1h
                        nc.vector.tensor_scalar_add(p1h_t[:, r0:r0 + rr, :],
                                                    p1[:, 0:rr, :], b1h_t[:])

            # phase-A psum/xk pools released -> conv2b gets 7 PSUM banks
            _ph_a.close()
            ps2p = ExitStack()
            ps2 = ps2p.enter_context(
                tc.tile_pool(name=f"ps2{_rep}", bufs=8, space="PSUM"))

            # ---- Phase B: conv2b, fp8 DoubleRow, weights outermost over
            # groups of 5 row-blocks (5 live PSUM accumulators)
            def buf_rhs(r, rr, kw):
                # [128, 2, rr*100] flat view of buf rows r..r+rr-1, col
                # offset kw; reads beyond row r+rr-1 land in later rows /
                # the scratch row and only feed garbage output columns
                sl = buf_t[:, :, r:r + rr, :]
                return AP(tensor=sl.tensor, offset=sl.offset + kw,
                          ap=[sl.ap[0], sl.ap[1], [1, rr * 100]])

            for g, (b0, nb) in enumerate(((0, 8), (8, 8), (16, 4))):
                h0s = [5 * (b0 + q) for q in range(nb)]
                rrs = [min(RB, H - h0) for h0 in h0s]
                p2s = [ps2.tile([128, RB, 100], F32, tag="p2",
                                name=f"p2_{g}_{q}") for q in range(nb)]
                for j in range(25):
                    kh, kw = divmod(j, 5)
                    # one explicit weight load per (kh,kw); the matmuls
                    # below are marked non-self-loading so the 256-column
                    # DoubleRow LDWEIGHTS is paid once, not nb times
                    nc.tensor.ldweights(w2bd_t[:, j], perf_mode=DR)
                    for q in range(nb):
                        h0, rr = h0s[q], rrs[q]
                        inst = nc.tensor.matmul(
                            p2s[q][:, 0:rr, :], w2bd_t[:, j],
                            buf_rhs(h0 + kh, rr, kw),
                            start=(j == 0), stop=(j == 24), perf_mode=DR)
                        raw = inst.ins if hasattr(inst, "ins") else inst
                        raw.ldweights = False
                for q in range(nb):
                    h0, rr = h0s[q], rrs[q]
                    # w2b/b2b pre-halved; w2b also x32 for fp8 -> scale back
                    tt = tmp.tile([CO, RB, W], F32)
                    nc.scalar.activation(tt[:, 0:rr, :],
                                         p2s[q][0:CO, 0:rr, 0:W], Relu,
                                         bias=b2bh_t[:], scale=1.0 / W2B_SCALE)
                    ot = outp.tile([CO, RB, W], F32)
                    nc.vector.tensor_add(ot[:, 0:rr, :], tt[:, 0:rr, :],
                                         p1h_t[:, h0:h0 + rr, :])
                    nc.scalar.dma_start(out=out[:, h0:h0 + rr, :],
                                        in_=ot[:, 0:rr, :])
            ps2p.close()

    nc.compile()
    return nc


def _prep_in_maps(pic, w1, b1, w2a, b2a, w2b, b2b):
    pic = np.asarray(pic, dtype=np.float32).reshape(B, 25, H, W)
    w1r = np.asarray(w1, dtype=np.float32).reshape(400, 25, 9, 9)
    b1 = np.asarray(b1, dtype=np.float32)
    w2a = np.asarray(w2a, dtype=np.float32)
    b2a = np.asarray(b2a, dtype=np.float32)
    w2b = np.asarray(w2b, dtype=np.float32)
    b2b = np.asarray(b2b, dtype=np.float32)

    xpad = np.full((B, 25, HP, WP), 0.5, dtype=np.float32)
    xpad[:, :, 4:4 + H, 4:4 + W] = pic
    # xk1[b, cin*5+kh, r, w] = xpad[b, cin, r+kh, w]   (kh 0..4, r 0..99)
    xk1 = np.stack([xpad[:, :, kh:kh + 100, :] for kh in range(5)],
                   axis=2).reshape(B, 125, 100, WP)
    # xk2[b, cin*4+kh', h, w] = xpad[b, cin, h+5+kh', w] (kh' 0..3, h 0..95)
    xk2 = np.stack([xpad[:, :, 5 + kh:5 + kh + 96, :] for kh in range(4)],
                   axis=2).reshape(B, 100, 96, WP)

    # W2A[p=(a1*5+a2)*5+kh, kw, m=a1'*60+a2'*20+c] = w2a[c,0,da1,da2,kh,kw]
    W2A = np.zeros((125, 5, 180), dtype=np.float32)
    for a1p in range(3):
        for a2p in range(3):
            m0 = a1p * 60 + a2p * 20
            for da1 in range(3):
                for da2 in range(3):
                    p0 = ((a1p + da1) * 5 + (a2p + da2)) * 5
                    W2A[p0:p0 + 5, :, m0:m0 + 20] = np.transpose(
                        w2a[:, 0, da1, da2, :, :], (1, 2, 0))
    ba_full = np.tile(b2a, 9).astype(np.float32)[:, None]  # [180,1]

    in_maps = []
    for core in range(8):
        b, cc = divmod(core, 4)
        co0 = cc * CO
        # w1, w2b (and their biases) pre-scaled by 0.5 so the (p1+p2)/2
        # average is folded into the matmuls.
        w1sl = 0.5 * w1r[co0:co0 + CO]  # [100, 25, 9, 9]
        W1A = np.ascontiguousarray(
            np.transpose(w1sl[:, :, 0:5, :], (1, 2, 3, 0)).reshape(125, 9, CO))
        W1B = np.ascontiguousarray(
            np.transpose(w1sl[:, :, 5:9, :], (1, 2, 3, 0)).reshape(100, 9, CO))
        w2bsl = 0.5 * w2b[co0:co0 + CO]  # [100, 20, 3, 3, 5, 5]
        # [sigma=(da1,da2,c), (kh,kw), m]
        W2B = np.transpose(w2bsl, (2, 3, 1, 4, 5, 0)).reshape(180, 25, CO)
        w2bd = np.zeros((128, 25, 2, 128), dtype=np.float32)
        w2bd[:, :, 0, 0:CO] = W2B_SCALE * W2B[0:128]
        w2bd[0:52, :, 1, 0:CO] = W2B_SCALE * W2B[128:180]
        in_maps.append({
            "xk1": np.ascontiguousarray(xk1[b].astype(MM_NP)),
            "xk2": np.ascontiguousarray(xk2[b].astype(MM_NP)),
            "w1a": W1A.astype(MM_NP),
            "w1b": W1B.astype(MM_NP),
            "w2a": np.ascontiguousarray(W2A.astype(MM_NP)),
            "w2bd": np.ascontiguousarray(w2bd.astype(NP8)),
            "ba1": np.ascontiguousarray(ba_full[:128]),
            "ba2": np.ascontiguousarray(ba_full[128:]),
            "b1h": np.ascontiguousarray((0.5 * b1[co0:co0 + CO])[:, None]),
            "b2bh": np.ascontiguousarray((0.5 * b2b[co0:co0 + CO])[:, None]),
        })
    return in_maps


def _get_runner(reps=1):
    """Build nc once per reps and return a cached jitted SPMD executor."""
    if reps in _RUNNERS:
        return _RUNNERS[reps]

    import jax
    from jax.experimental.shard_map import shard_map
    from jax.sharding import Mesh, NamedSharding, PartitionSpec

    from concourse import mybir as _mybir
    from concourse.bass2jax import (_bass_exec_p, install_neuronx_cc_hook,
                                    partition_id_tensor)

    nc = _build_nc(reps)
    install_neuronx_cc_hook()

    n_cores = 8
    partition_name = (nc.partition_id_tensor.name
                      if nc.partition_id_tensor else None)
    in_names, out_names, out_avals, zero_outs = [], [], [], []
    for alloc in nc.m.functions[0].allocations:
        if not isinstance(alloc, _mybir.MemoryLocationSet):
            continue
        name = alloc.memorylocations[0].name
        if alloc.kind == "ExternalInput":
            if name != partition_name:
                in_names.append(name)
        elif alloc.kind == "ExternalOutput":
            shape = tuple(alloc.tensor_shape)
            dtype = _mybir.dt.np(alloc.dtype)
            out_names.append(name)
            out_avals.append(jax.core.ShapedArray(shape, dtype))
            zero_outs.append(np.zeros((n_cores * shape[0],) + shape[1:], dtype))
    assert nc.dbg_addr is None
    n_params = len(in_names)
    all_names = in_names + out_names
    if partition_name is not None:
        all_names = all_names + [partition_name]

    def _body(*args):
        operands = list(args)
        if partition_name is not None:
            operands.append(partition_id_tensor())
        outs = _bass_exec_p.bind(
            *operands,
            out_avals=tuple(out_avals),
            in_names=tuple(all_names),
            out_names=tuple(out_names),
            lowering_input_output_aliases=(),
            sim_require_finite=True,
            sim_require_nnan=True,
            nc=nc,
        )
        return tuple(outs)

    devices = jax.devices()[:n_cores]
    mesh = Mesh(np.asarray(devices), ("core",))
    nspec = (PartitionSpec("core"),) * (n_params + len(out_names))
    sharded = jax.jit(
        shard_map(_body, mesh=mesh, in_specs=nspec,
                  out_specs=(PartitionSpec("core"),) * len(out_names)),
        keep_unused=True)
    sharding = NamedSharding(mesh, PartitionSpec("core"))

    class Runner:
        def put(self, in_maps):
            """Transfer inputs (+ zero output bufs) to the devices once."""
            concat_in = [
                np.concatenate([np.asarray(m[name]) for m in in_maps], axis=0)
                for name in in_names
            ]
            return [jax.device_put(x, sharding)
                    for x in concat_in + zero_outs]

        def exec_timed(self, dev_args):
            t0 = time.perf_counter()
            out_arrs = sharded(*dev_args)
            # one sync only: under axon each block_until_ready is a costly
            # RPC, and blocking any output waits for the whole execution
            out_arrs[0].block_until_ready()
            return out_arrs, time.perf_counter() - t0

        def __call__(self, in_maps):
            out_arrs, dt = self.exec_timed(self.put(in_maps))
            per_core = [
                {name: np.asarray(out_arrs[i]).reshape(
                    n_cores, *out_avals[i].shape)[c]
                 for i, name in enumerate(out_names)}
                for c in range(n_cores)
            ]
            return per_core, dt

    run = Runner()
    _RUNNERS[reps] = run
    return run


def kernel(pic, w1, b1, w2a, b2a, w2b, b2b):
    run = _get_runner()
    in_maps = _prep_in_maps(pic, w1, b1, w2a, b2a, w2b, b2b)
    results, _ = run(in_maps)

    mid = np.empty((B, 400, H, W), dtype=np.float32)
    for core in range(8):
        b, cc = divmod(core, 4)
        mid[b, cc * CO:(cc + 1) * CO] = results[core]["out"]
    # pixel shuffle r=4, then split 25 -> 5x5
    y = mid.reshape(B, 25, 4, 4, H, W).transpose(0, 1, 4, 2, 5, 3)
    return np.ascontiguousarray(y).reshape(B, 5, 5, H * 4, W * 4)
